# revision 18
# baseline (speedup 1.0000x reference)
"""CompGCN message-passing kernel for 8 Trainium2 NeuronCores (Bass/Tile).

Strategy
--------
* Edges are sorted by dst and sharded across 8 cores by contiguous node
  ranges (6250 nodes/core, padded to 6272 = 49 ranges of 128).
* Linearity: segment_sum(msg @ W) == segment_sum(msg) @ W, so the heavy
  per-edge matmul with w_in collapses to a per-node matmul after
  aggregation.  Per-edge work is gather + elementwise mul + scatter-add.
* edge_attr layers are recomputed from ea0 with composed matrices
  M_i = W_rel[0].T @ ... @ W_rel[i-1].T (host-folded weights).
* Scatter-add is a TensorE matmul with host-built one-hot S chunks
  (lhsT = S [128e x 128slot], rhs = u [128e x 128d]) accumulating into a
  PSUM tile per 128-node range; the exact f32 1/deg mean is fused into
  the ACT evacuation as a per-partition scale.
* Node-side state is kept d-major ([d x node]) so BN stats are free-dim
  reductions and BN+bias+ReLU fuse into one ACT op; cross-core BN stats
  go through a tiny AllReduce, and the bf16 node-major gather table is
  rebuilt each layer with PE transposes + an AllGather.
"""

import math
import sys

import numpy as np

sys.path.insert(0, "/opt/trn_rl_repo")

import ml_dtypes

BF16 = ml_dtypes.bfloat16

N, E, D, L = 50000, 640000, 128, 3
EPS = 1e-5
NCORES = 8
P = 128
NPC = N // NCORES            # 6250 real nodes per core
RANGES = 49                  # 128-node ranges per core
NPAD = RANGES * P            # 6272 padded nodes per core
TBL = NCORES * NPAD          # gather-table rows (AllGather layout)
GATHER_K = 16                # chunks per gather batch
GRP = 4                      # chunks per DVE-mul / ACT-evac group
NT = 512                     # node-phase tile width


# ----------------------------------------------------------------------------
# Host preprocessing: sorting, sharding, one-hot scatter matrices, weights.
# ----------------------------------------------------------------------------

def _preprocess(x, edge_attr, w_self, b_self, w_in, b_in, w_rel, b_rel,
                bn_gamma, bn_beta, edge_index):
    src = np.asarray(edge_index[0], dtype=np.int64)
    dst = np.asarray(edge_index[1], dtype=np.int64)
    x = np.asarray(x, dtype=np.float32)
    ea = np.asarray(edge_attr, dtype=np.float32)

    core = dst // NPC
    local = dst - core * NPC          # 0..NPC-1
    rng = local // P                  # 0..RANGES-1 (only 0..48 reachable)

    cnt_cr = np.zeros((NCORES, RANGES), np.int64)
    np.add.at(cnt_cr, (core, rng), 1)
    K_r = np.maximum(1, -(-cnt_cr.max(axis=0) // P))     # chunks per range
    C = int(K_r.sum())
    chunk0 = np.concatenate([[0], np.cumsum(K_r)[:-1]])  # first chunk of range
    range_of_chunk = np.repeat(np.arange(RANGES), K_r)

    # ag-layout position of every node
    ag_pos_all = (np.arange(N) // NPC) * NPAD + (np.arange(N) % NPC)

    order = np.lexsort((local, core))  # edges sorted by (core, local dst)
    core_s, local_s, rng_s = core[order], local[order], rng[order]
    src_s = src[order]

    pres = []
    for c in range(NCORES):
        sel = np.nonzero(core_s == c)[0]
        lc, rc = local_s[sel], rng_s[sel]
        eids = order[sel]              # original edge ids, dst-sorted
        nec = len(sel)

        # slot position of each edge inside the padded chunk structure
        # edges of range r occupy chunk0[r]*P ... in order
        cnt_r = np.bincount(rc, minlength=RANGES)
        off_r = np.concatenate([[0], np.cumsum(cnt_r)[:-1]])
        pos_in_range = np.arange(nec) - off_r[rc]
        pos = chunk0[rc] * P + pos_in_range          # padded flat position

        CP = C * P
        src_pos = np.zeros(CP, np.int32)             # dummy -> row 0
        slot_oh = np.full(CP, -1, np.int64)          # -1 -> no one-hot
        orig_eid = np.full(CP, -1, np.int64)
        src_pos[pos] = ag_pos_all[src_s[sel]].astype(np.int32)
        slot_oh[pos] = lc - rc * P                   # 0..127 within range
        orig_eid[pos] = eids

        S = np.zeros((CP, P), np.float32)
        valid = slot_oh >= 0
        S[np.nonzero(valid)[0], slot_oh[valid]] = 1.0

        ea_sorted = np.zeros((CP, D), np.float32)
        ea_sorted[pos] = ea[eids]

        cnt_node = np.bincount(lc, minlength=NPAD).astype(np.float32)
        inv_cnt = 1.0 / np.maximum(cnt_node, 1.0)

        ones_row = np.zeros(NPAD, np.float32)
        ones_row[:NPC] = 1.0
        mask_row = (cnt_node > 0).astype(np.float32)
        mask_row[NPC:] = 0.0

        pres.append(dict(
            idx=np.ascontiguousarray(
                src_pos.reshape(C, P).T).astype(np.int32),      # [P, C]
            ea_l0=ea_sorted.astype(BF16),                       # [C*P, D]
            ea0T=np.ascontiguousarray(
                ea_sorted.reshape(C, P, D).transpose(0, 2, 1)).astype(BF16),
            S=np.ascontiguousarray(S.reshape(C, P, P)).astype(BF16),
            invcnt=np.ascontiguousarray(
                inv_cnt.reshape(RANGES, P).T).astype(np.float32),  # [P, RANGES]
            bias_rhs=np.stack([ones_row, mask_row]).astype(np.float32),
            xshard=np.concatenate(
                [x[c * NPC:(c + 1) * NPC],
                 np.zeros((NPAD - NPC, D), np.float32)]),       # [NPAD, D]
            orig_eid=orig_eid,
        ))

    # replicated bf16 gather table for layer 0 (AllGather layout)
    xt0 = np.zeros((TBL, D), np.float32)
    for c in range(NCORES):
        xt0[c * NPAD:c * NPAD + NPC] = x[c * NPC:(c + 1) * NPC]
    xt0 = xt0.astype(BF16)

    # composed edge_attr transform weights (f64 for accuracy)
    w_rel64 = np.asarray(w_rel, dtype=np.float64)
    b_rel64 = np.asarray(b_rel, dtype=np.float64)
    M = np.zeros((L, D, D))
    cvec = np.zeros((L, D))
    Mi = np.eye(D)
    ci = np.zeros(D)
    for i in range(L):
        Mi = Mi @ w_rel64[i].T
        ci = ci @ w_rel64[i].T + b_rel64[i]
        M[i] = Mi
        cvec[i] = ci

    w_in = np.asarray(w_in, dtype=np.float32)
    w_self = np.asarray(w_self, dtype=np.float32)
    shared = dict(
        xt0=xt0,
        winT=np.ascontiguousarray(w_in.transpose(0, 2, 1)),
        wselfT=np.ascontiguousarray(w_self.transpose(0, 2, 1)),
        bias_lhs=np.stack([np.asarray(b_self, np.float32),
                           np.asarray(b_in, np.float32)], axis=1),  # [L,2,D]
        M=M.astype(BF16),
        cvec=cvec.astype(np.float32).reshape(L, 1, D),
        bng=np.ascontiguousarray(np.asarray(bn_gamma, np.float32).T),  # [D,L-1]
        bnb=np.ascontiguousarray(np.asarray(bn_beta, np.float32).T),
    )
    cfg = dict(C=C, K_r=K_r, chunk0=chunk0, range_of_chunk=range_of_chunk,
               c_nonzero=[bool(np.any(cvec[i] != 0)) for i in range(L)])
    return cfg, shared, pres


# ----------------------------------------------------------------------------
# Numpy emulator of the device computation (same preprocessed arrays).
# Used for fast validation of the indexing/layout logic.
# ----------------------------------------------------------------------------

def _emulate(cfg, shared, pres):
    C = cfg["C"]
    roc = cfg["range_of_chunk"]
    table = np.asarray(shared["xt0"], np.float32)
    xT = [np.asarray(p["xshard"], np.float32).T.copy() for p in pres]  # [D,NPAD]
    agg = [None] * NCORES
    ea_out = [None] * NCORES
    inv_n = 1.0 / N

    for layer in range(L):
        # edge phase
        for c in range(NCORES):
            pre = pres[c]
            idx = pre["idx"]                       # [P, C]
            xg = table[idx.T.reshape(-1)]          # [C*P, D] f32 (bf16 values)
            if layer == 0:
                eaL = np.asarray(pre["ea_l0"], np.float32)
            else:
                ea0T = np.asarray(pre["ea0T"], np.float32)  # [C, D, P]
                Mb = np.asarray(shared["M"][layer - 1], np.float32)
                eaL = np.einsum("cdp,de->cpe", ea0T, Mb).reshape(C * P, D)
                eaL += shared["cvec"][layer - 1][0]
                eaL = eaL.astype(BF16).astype(np.float32)
            u = (xg * eaL).astype(BF16).astype(np.float32)
            S = np.asarray(pre["S"], np.float32)   # [C, P, P]
            agg_nm = np.zeros((NPAD, D), np.float32)
            for r in range(RANGES):
                k0, k1 = cfg["chunk0"][r], cfg["chunk0"][r] + cfg["K_r"][r]
                part = np.einsum("kes,ked->sd", S[k0:k1],
                                 u.reshape(C, P, D)[k0:k1])
                agg_nm[r * P:(r + 1) * P] = part
            inv_cnt = pre["invcnt"].T.reshape(-1)  # [NPAD]
            agg[c] = (agg_nm * inv_cnt[:, None]).T  # [D, NPAD] d-major
            if layer == L - 1:
                ea0T = np.asarray(pre["ea0T"], np.float32)
                Mb = np.asarray(shared["M"][L - 1], np.float32)
                eo = np.einsum("cdp,de->cpe", ea0T, Mb).reshape(C * P, D)
                eo += shared["cvec"][L - 1][0]
                ea_out[c] = eo

        # node phase
        stats = np.zeros((D, 2), np.float32)
        h = [None] * NCORES
        for c in range(NCORES):
            pre = pres[c]
            winT = shared["winT"][layer]
            wselfT = shared["wselfT"][layer]
            blhs = shared["bias_lhs"][layer]       # [2, D]
            brhs = pre["bias_rhs"]                 # [2, NPAD]
            hT = winT.T @ agg[c] + wselfT.T @ xT[c] + blhs.T @ brhs
            h[c] = hT
            stats[:, 0] += hT.sum(axis=1)
            stats[:, 1] += (hT * hT).sum(axis=1)
        if layer < L - 1:
            mean = stats[:, 0] * inv_n
            var = stats[:, 1] * inv_n - mean * mean
            s = shared["bng"][:, layer] / np.sqrt(var + EPS)
            t = shared["bnb"][:, layer] - mean * s
            table = np.zeros((TBL, D), np.float32)
            for c in range(NCORES):
                xn = np.maximum(h[c] * s[:, None] + t[:, None], 0.0)
                xn[:, NPC:] = 0.0
                xT[c] = xn
                table[c * NPAD:c * NPAD + NPAD] = \
                    xn.T.astype(BF16).astype(np.float32)
        else:
            for c in range(NCORES):
                xT[c] = h[c]

    x_full = np.zeros((N, D), np.float32)
    ea_full = np.zeros((E, D), np.float32)
    for c in range(NCORES):
        x_full[c * NPC:(c + 1) * NPC] = xT[c].T[:NPC]
        eid = pres[c]["orig_eid"]
        v = eid >= 0
        ea_full[eid[v]] = ea_out[c][v]
    return x_full, ea_full


# ----------------------------------------------------------------------------
# Bass kernel builder.
# ----------------------------------------------------------------------------

def _build_bass(cfg):
    from contextlib import ExitStack

    from concourse import bacc, bass, mybir
    from concourse import tile as tile_mod
    from concourse.masks import make_identity

    dt = mybir.dt
    C = cfg["C"]
    K_r = cfg["K_r"]
    chunk0 = cfg["chunk0"]
    roc = cfg["range_of_chunk"]
    nB = -(-C // GATHER_K)

    nc = bacc.Bacc(None, num_devices=NCORES)

    # ---- I/O -----------------------------------------------------------
    xt0 = nc.dram_tensor("xt0", [TBL, D], dt.bfloat16, kind="ExternalInput")
    xshard = nc.dram_tensor("xshard", [NPAD, D], dt.float32, kind="ExternalInput")
    idx_d = nc.dram_tensor("idx", [P, C], dt.int32, kind="ExternalInput")
    eal0_d = nc.dram_tensor("ea_l0", [C * P, D], dt.bfloat16, kind="ExternalInput")
    ea0T_d = nc.dram_tensor("ea0T", [C, D, P], dt.bfloat16, kind="ExternalInput")
    S_d = nc.dram_tensor("S", [C, P, P], dt.bfloat16, kind="ExternalInput")
    invc_d = nc.dram_tensor("invcnt", [P, RANGES], dt.float32, kind="ExternalInput")
    brhs_d = nc.dram_tensor("bias_rhs", [2, NPAD], dt.float32, kind="ExternalInput")
    winT_d = nc.dram_tensor("winT", [L, D, D], dt.float32, kind="ExternalInput")
    wselfT_d = nc.dram_tensor("wselfT", [L, D, D], dt.float32, kind="ExternalInput")
    blhs_d = nc.dram_tensor("bias_lhs", [L, 2, D], dt.float32, kind="ExternalInput")
    M_d = nc.dram_tensor("M", [L, D, D], dt.bfloat16, kind="ExternalInput")
    c_d = nc.dram_tensor("cvec", [L, 1, D], dt.float32, kind="ExternalInput")
    bng_d = nc.dram_tensor("bng", [D, L - 1], dt.float32, kind="ExternalInput")
    bnb_d = nc.dram_tensor("bnb", [D, L - 1], dt.float32, kind="ExternalInput")
    xout_d = nc.dram_tensor("x_out", [NPAD, D], dt.float32, kind="ExternalOutput")
    dbg_xg = nc.dram_tensor("dbg_xg", [P, GATHER_K * D], dt.bfloat16,
                            kind="ExternalOutput")
    dbg_aggT = nc.dram_tensor("dbg_aggT", [D, NPAD], dt.float32,
                              kind="ExternalOutput")
    dbg_h = nc.dram_tensor("dbg_h", [D, NPAD], dt.float32,
                           kind="ExternalOutput")
    dbg_stats = nc.dram_tensor("dbg_stats", [D, 4], dt.float32,
                               kind="ExternalOutput")
    eaout_d = nc.dram_tensor("ea_out", [C * P, D], dt.float32, kind="ExternalOutput")

    # e-major views of per-edge DRAM arrays: [P, nchunk, D] 3D APs
    eal0_v = eal0_d[:].rearrange("(c p) d -> p c d", p=P)
    ea0T_v = ea0T_d[:].rearrange("c d p -> d c p")
    S_v = S_d[:].rearrange("c p s -> p c s")
    eaout_v = eaout_d[:].rearrange("(c p) d -> p c d", p=P)

    with ExitStack() as ctx:
        tc = ctx.enter_context(tile_mod.TileContext(nc))
        sb = ctx.enter_context(tc.tile_pool(name="sb", bufs=1))
        ps = ctx.enter_context(tc.tile_pool(name="ps", bufs=1, space="PSUM"))
        dram = ctx.enter_context(tc.tile_pool(name="dram", bufs=1, space="DRAM"))

        # ---- persistent small tiles -----------------------------------
        ident = sb.tile([P, P], dt.float32, tag="ident")
        make_identity(nc, ident)
        # transpose-mode matmuls lower to a single S3_LW with one wait slot,
        # which overflows under Tile deps -> use plain matmul against the
        # identity for transposes (out = lhsT.T @ I).
        winT_sb = sb.tile([D, L * D], dt.float32, tag="winT", name="winT_sb")
        nc.sync.dma_start(out=winT_sb.rearrange("d (l f) -> d l f", l=L),
                          in_=winT_d[:].rearrange("l d f -> d l f"))
        wselfT_sb = sb.tile([D, L * D], dt.float32, tag="wselfT",
                            name="wselfT_sb")
        nc.sync.dma_start(out=wselfT_sb.rearrange("d (l f) -> d l f", l=L),
                          in_=wselfT_d[:].rearrange("l d f -> d l f"))
        M_sb = sb.tile([D, L * D], dt.bfloat16, tag="M")
        nc.sync.dma_start(out=M_sb.rearrange("d (l e) -> d l e", l=L),
                          in_=M_d[:].rearrange("l d e -> d l e"))
        cvec_sb = sb.tile([1, L * D], dt.float32, tag="cvec")
        nc.sync.dma_start(out=cvec_sb.rearrange("o (l d) -> o l d", l=L),
                          in_=c_d[:].rearrange("l o d -> o l d"))
        blhs_sb = sb.tile([2, L * D], dt.float32, tag="blhs")
        nc.sync.dma_start(out=blhs_sb.rearrange("t (l d) -> t l d", l=L),
                          in_=blhs_d[:].rearrange("l t d -> t l d"))
        brhs_sb = sb.tile([2, NPAD], dt.float32, tag="brhs")
        nc.sync.dma_start(out=brhs_sb, in_=brhs_d[:])
        invc_sb = sb.tile([P, RANGES], dt.float32, tag="invc")
        nc.sync.dma_start(out=invc_sb, in_=invc_d[:])
        bng_sb = sb.tile([D, L - 1], dt.float32, tag="bng")
        nc.sync.dma_start(out=bng_sb, in_=bng_d[:])
        bnb_sb = sb.tile([D, L - 1], dt.float32, tag="bnb")
        nc.sync.dma_start(out=bnb_sb, in_=bnb_d[:])
        ones_col = sb.tile([1, P], dt.float32, tag="ones_col")
        nc.vector.memset(ones_col, 1.0)

        # ---- persistent big state -------------------------------------
        xT = sb.tile([D, NPAD], dt.float32, tag="xT", bufs=2)
        aggT = sb.tile([D, NPAD], dt.float32, tag="aggT", bufs=1)
        hT = sb.tile([D, NPAD], dt.float32, tag="hT", bufs=1)

        # initial transpose of x shard -> xT (f32, d-major)
        for t in range(RANGES):
            xnm = sb.tile([P, D], dt.float32, tag="xnm", bufs=3)
            nc.sync.dma_start(out=xnm, in_=xshard[t * P:(t + 1) * P, :])
            tp = ps.tile([P, P], dt.float32, tag="tp", bufs=2, space="PSUM")
            nc.tensor.matmul(out=tp[:], lhsT=xnm[:], rhs=ident[:],
                             start=True, stop=True)
            nc.scalar.copy(out=xT[:, t * P:(t + 1) * P], in_=tp[:])

        gtables = [xt0[:]]  # gather table per layer

        node_tiles = []
        o = 0
        while o < NPAD:
            w = min(NT, NPAD - o)
            node_tiles.append((o, w))
            o += w

        for layer in range(L):
            table = gtables[layer]
            last = layer == L - 1

            # ================= edge phase ===============================
            for b in range(nB):
                c0 = b * GATHER_K
                kk = min(GATHER_K, C - c0)
                idx_sb = sb.tile([P, GATHER_K], dt.int32, tag="idx", bufs=3)
                nc.sync.dma_start(out=idx_sb[:, :kk],
                                  in_=idx_d[:, c0:c0 + kk])
                xg = sb.tile([P, GATHER_K * D], dt.bfloat16, tag="xg", bufs=3)
                # HW indirect DMA consumes one index per dest partition-run:
                # gather one 128-edge chunk per instruction.
                for j in range(kk):
                    nc.gpsimd.indirect_dma_start(
                        out=xg[:, j * D:(j + 1) * D],
                        out_offset=None,
                        in_=table,
                        in_offset=bass.IndirectOffsetOnAxis(
                            ap=idx_sb[:, j:j + 1], axis=0),
                    )
                if layer == 0 and b == 0:
                    nc.sync.dma_start(out=dbg_xg[:], in_=xg[:])
                if layer == 0:
                    eab = sb.tile([P, GATHER_K * D], dt.bfloat16, tag="eab", bufs=3)
                    nc.sync.dma_start(
                        out=eab[:, :kk * D].rearrange("p (c d) -> p c d", c=kk),
                        in_=eal0_v[:, c0:c0 + kk, :])
                else:
                    e0T = sb.tile([D, GATHER_K * P], dt.bfloat16, tag="e0T", bufs=3)
                    nc.sync.dma_start(
                        out=e0T[:, :kk * P].rearrange("d (c p) -> d c p", c=kk),
                        in_=ea0T_v[:, c0:c0 + kk, :])
                Ssb = sb.tile([P, GATHER_K * P], dt.bfloat16, tag="Ssb", bufs=3)
                nc.sync.dma_start(
                    out=Ssb[:, :kk * P].rearrange("p (c s) -> p c s", c=kk),
                    in_=S_v[:, c0:c0 + kk, :])

                for g0 in range(0, kk, GRP):
                    gw = min(GRP, kk - g0)
                    if layer > 0:
                        eps_t = ps.tile([P, GRP * D], dt.float32, tag="ps512",
                                        bufs=2, space="PSUM")
                        if last:
                            ops_t = ps.tile([P, GRP * D], dt.float32,
                                            tag="ops", bufs=2, space="PSUM")
                        for j in range(gw):
                            cid = c0 + g0 + j
                            lhs = e0T[:, (g0 + j) * P:(g0 + j + 1) * P]
                            nc.tensor.matmul(
                                out=eps_t[:, j * D:(j + 1) * D], lhsT=lhs,
                                rhs=M_sb[:, (layer - 1) * D:layer * D],
                                start=True, stop=not cfg["c_nonzero"][layer - 1])
                            if cfg["c_nonzero"][layer - 1]:
                                nc.tensor.matmul(
                                    out=eps_t[:, j * D:(j + 1) * D],
                                    lhsT=ones_col[:, :P],
                                    rhs=cvec_sb[:, (layer - 1) * D:layer * D],
                                    start=False, stop=True)
                            if last:
                                nc.tensor.matmul(
                                    out=ops_t[:, j * D:(j + 1) * D], lhsT=lhs,
                                    rhs=M_sb[:, (L - 1) * D:L * D],
                                    start=True, stop=not cfg["c_nonzero"][L - 1])
                                if cfg["c_nonzero"][L - 1]:
                                    nc.tensor.matmul(
                                        out=ops_t[:, j * D:(j + 1) * D],
                                        lhsT=ones_col[:, :P],
                                        rhs=cvec_sb[:, (L - 1) * D:L * D],
                                        start=False, stop=True)
                        eab = sb.tile([P, GRP * D], dt.bfloat16, tag="eab2", bufs=4)
                        nc.scalar.copy(out=eab[:, :gw * D], in_=eps_t[:, :gw * D])
                        ea_sl = eab[:, :gw * D]
                        if last:
                            eo_sb = sb.tile([P, GRP * D], dt.float32,
                                            tag="eo", bufs=3)
                            nc.scalar.copy(out=eo_sb[:, :gw * D],
                                           in_=ops_t[:, :gw * D])
                            nc.sync.dma_start(
                                out=eaout_v[:, c0 + g0:c0 + g0 + gw, :],
                                in_=eo_sb[:, :gw * D].rearrange(
                                    "p (c d) -> p c d", c=gw))
                    else:
                        ea_sl = eab[:, g0 * D:(g0 + gw) * D]

                    u = sb.tile([P, GRP * D], dt.bfloat16, tag="u", bufs=4)
                    nc.vector.tensor_mul(u[:, :gw * D],
                                         xg[:, (g0) * D:(g0 + gw) * D], ea_sl)

                    for j in range(gw):
                        cid = c0 + g0 + j
                        r = int(roc[cid])
                        first = cid == chunk0[r]
                        lastc = cid == chunk0[r] + K_r[r] - 1
                        if first:
                            agg_ps = ps.tile([P, D], dt.float32, tag="agg",
                                             bufs=2, space="PSUM")
                            cfg.setdefault("_aggps", {})[r] = agg_ps
                        agg_ps = cfg["_aggps"][r]
                        nc.tensor.matmul(
                            out=agg_ps[:], lhsT=Ssb[:, (g0 + j) * P:(g0 + j + 1) * P],
                            rhs=u[:, j * D:(j + 1) * D],
                            start=first, stop=lastc)
                        if lastc:
                            # evacuate with exact 1/deg scale, node-major
                            agg_nm = sb.tile([P, D], dt.float32, tag="aggnm",
                                             bufs=3)
                            nc.scalar.activation(
                                out=agg_nm[:], in_=agg_ps[:],
                                func=mybir.ActivationFunctionType.Copy,
                                scale=invc_sb[:, r:r + 1])
                            tp2 = ps.tile([P, P], dt.float32, tag="tp",
                                          bufs=2, space="PSUM")
                            nc.tensor.matmul(out=tp2[:], lhsT=agg_nm[:],
                                             rhs=ident[:], start=True,
                                             stop=True)
                            nc.scalar.copy(out=aggT[:, r * P:(r + 1) * P],
                                           in_=tp2[:])

            if layer == 0:
                nc.sync.dma_start(out=dbg_aggT[:], in_=aggT[:])
            # ================= node phase ===============================
            nt = len(node_tiles)
            hsum = sb.tile([D, nt], dt.float32, tag="hsum")
            hsq = sb.tile([D, nt], dt.float32, tag="hsq")
            for ti, (o, w) in enumerate(node_tiles):
                hp = ps.tile([P, NT], dt.float32, tag="ps512", bufs=2, space="PSUM")
                nc.tensor.matmul(out=hp[:, :w],
                                 lhsT=winT_sb[:, layer * D:(layer + 1) * D],
                                 rhs=aggT[:, o:o + w],
                                 start=True, stop=False)
                nc.tensor.matmul(out=hp[:, :w],
                                 lhsT=wselfT_sb[:, layer * D:(layer + 1) * D],
                                 rhs=xT[:, o:o + w],
                                 start=False, stop=False)
                nc.tensor.matmul(out=hp[:, :w],
                                 lhsT=blhs_sb[:, layer * D:(layer + 1) * D],
                                 rhs=brhs_sb[:, o:o + w],
                                 start=False, stop=True)
                if not last:
                    nc.scalar.activation(
                        out=hT[:, o:o + w], in_=hp[:, :w],
                        func=mybir.ActivationFunctionType.Copy,
                        accum_out=hsum[:, ti:ti + 1])
                    sq_scr = sb.tile([D, NT], dt.float32, tag="sqscr", bufs=2)
                    nc.scalar.activation(
                        out=sq_scr[:, :w], in_=hT[:, o:o + w],
                        func=mybir.ActivationFunctionType.Square,
                        accum_out=hsq[:, ti:ti + 1])
                else:
                    nc.scalar.copy(out=hT[:, o:o + w], in_=hp[:, :w])

            if not last:
                # cross-core BN stats
                stats = sb.tile([D, 2], dt.float32, tag="stats", bufs=2)
                nc.vector.reduce_sum(stats[:, 0:1], hsum[:],
                                     axis=mybir.AxisListType.X)
                nc.vector.reduce_sum(stats[:, 1:2], hsq[:],
                                     axis=mybir.AxisListType.X)
                ar_in = dram.tile([D, 2], dt.float32, tag="arin", bufs=2)
                ar_out = dram.tile([D, 2], dt.float32, tag="arout", bufs=2,
                                   addr_space="Shared")
                nc.sync.dma_start(out=ar_in[:], in_=stats[:])
                nc.gpsimd.collective_compute(
                    "AllReduce", mybir.AluOpType.add,
                    replica_groups=[list(range(NCORES))],
                    ins=[ar_in[:]], outs=[ar_out[:]])
                gstats = sb.tile([D, 2], dt.float32, tag="gstats", bufs=2)
                nc.sync.dma_start(out=gstats[:], in_=ar_out[:])
                if layer == 0:
                    nc.sync.dma_start(out=dbg_h[:], in_=hT[:])
                    nc.sync.dma_start(out=dbg_stats[:, 0:2], in_=stats[:])
                    nc.sync.dma_start(out=dbg_stats[:, 2:4], in_=gstats[:])

                mean = sb.tile([D, 1], dt.float32, tag="mean", bufs=2)
                nc.scalar.mul(mean[:], gstats[:, 0:1], 1.0 / N)
                ex2 = sb.tile([D, 1], dt.float32, tag="ex2", bufs=2)
                nc.scalar.mul(ex2[:], gstats[:, 1:2], 1.0 / N)
                m2 = sb.tile([D, 1], dt.float32, tag="m2", bufs=2)
                nc.scalar.square(m2[:], mean[:])
                var = sb.tile([D, 1], dt.float32, tag="var", bufs=2)
                nc.vector.tensor_sub(var[:], ex2[:], m2[:])
                vare = sb.tile([D, 1], dt.float32, tag="vare", bufs=2)
                nc.vector.tensor_scalar_add(vare[:], var[:], float(EPS))
                std = sb.tile([D, 1], dt.float32, tag="std", bufs=2)
                nc.scalar.activation(std[:], vare[:],
                                     mybir.ActivationFunctionType.Sqrt)
                rs = sb.tile([D, 1], dt.float32, tag="rs", bufs=2)
                nc.vector.reciprocal(rs[:], std[:])
                svec = sb.tile([D, 1], dt.float32, tag="svec", bufs=2)
                nc.vector.tensor_mul(svec[:], rs[:], bng_sb[:, layer:layer + 1])
                ms = sb.tile([D, 1], dt.float32, tag="ms", bufs=2)
                nc.vector.tensor_mul(ms[:], mean[:], svec[:])
                tvec = sb.tile([D, 1], dt.float32, tag="tvec", bufs=2)
                nc.vector.tensor_sub(tvec[:], bnb_sb[:, layer:layer + 1], ms[:])

                xT = sb.tile([D, NPAD], dt.float32, tag="xT", bufs=2)
                for (o, w) in node_tiles:
                    nc.scalar.activation(
                        out=xT[:, o:o + w], in_=hT[:, o:o + w],
                        func=mybir.ActivationFunctionType.Relu,
                        bias=tvec[:, 0:1], scale=svec[:, 0:1])
                nc.vector.memset(xT[:, NPC:NPAD], 0.0)

                # rebuild bf16 node-major gather table: transpose + AllGather
                ag_in = dram.tile([NPAD, D], dt.bfloat16, tag="agin", bufs=2)
                ag_out = dram.tile([TBL, D], dt.bfloat16, tag="agout", bufs=2,
                                   addr_space="Shared")
                for t in range(RANGES):
                    tp3 = ps.tile([P, P], dt.float32, tag="tp", bufs=2,
                                  space="PSUM")
                    nc.tensor.matmul(out=tp3[:], lhsT=xT[:, t * P:(t + 1) * P],
                                     rhs=ident[:], start=True, stop=True)
                    xbf = sb.tile([P, D], dt.bfloat16, tag="xbf", bufs=3)
                    nc.scalar.copy(out=xbf[:], in_=tp3[:])
                    nc.sync.dma_start(out=ag_in[t * P:(t + 1) * P, :], in_=xbf[:])
                nc.gpsimd.collective_compute(
                    "AllGather", mybir.AluOpType.bypass,
                    replica_groups=[list(range(NCORES))],
                    ins=[ag_in[:]], outs=[ag_out[:]])
                gtables.append(ag_out[:])
                aggT = sb.tile([D, NPAD], dt.float32, tag="aggT", bufs=1)
                hT = sb.tile([D, NPAD], dt.float32, tag="hT", bufs=1)
            else:
                # final x output: transpose hT back to node-major f32
                for t in range(RANGES):
                    tp4 = ps.tile([P, P], dt.float32, tag="tp", bufs=2,
                                  space="PSUM")
                    nc.tensor.matmul(out=tp4[:], lhsT=hT[:, t * P:(t + 1) * P],
                                     rhs=ident[:], start=True, stop=True)
                    xo = sb.tile([P, D], dt.float32, tag="xo", bufs=3)
                    nc.scalar.copy(out=xo[:], in_=tp4[:])
                    nc.sync.dma_start(out=xout_d[t * P:(t + 1) * P, :], in_=xo[:])

    cfg.pop("_aggps", None)
    return nc


# ----------------------------------------------------------------------------
# Public entry point.
# ----------------------------------------------------------------------------

def _install_ntff_hook():
    """The agent image's antenv lacks axon_hooks; recreate it so
    run_bass_kernel_spmd(trace=True) can capture NTFF profiles."""
    import types

    try:
        import antenv
        try:
            from antenv.axon_hooks import get_axon_ntff_profile_hook
            return get_axon_ntff_profile_hook() is not None
        except ImportError:
            pass
        sys.path.insert(0, "/root/.axon_site")
        from trn_agent_boot.trn_boot import _ntff_profile_via_ctypes
        hook = _ntff_profile_via_ctypes("/opt/axon/libaxon_pjrt.so")
        if hook is None:
            return False
        mod = types.ModuleType("antenv.axon_hooks")
        mod._hook = hook
        mod.get_axon_ntff_profile_hook = lambda: mod._hook
        mod.set_axon_ntff_profile_hook = lambda h: setattr(mod, "_hook", h)
        sys.modules["antenv.axon_hooks"] = mod
        antenv.axon_hooks = mod
        return True
    except Exception as exc:  # degrade to untraced run
        print(f"NTFF hook install failed: {exc}")
        return False


def kernel(**inputs):
    cfg, shared, pres = _preprocess(**inputs)

    nc = _build_bass(cfg)
    nc.finalize()   # run Bacc passes (wait splitting, reg alloc, DCE)

    in_maps = []
    for c in range(NCORES):
        m = dict(shared)
        p = pres[c]
        m.update(idx=p["idx"], ea_l0=p["ea_l0"], ea0T=p["ea0T"], S=p["S"],
                 invcnt=p["invcnt"], bias_rhs=p["bias_rhs"],
                 xshard=p["xshard"])
        m = {k: np.ascontiguousarray(v) for k, v in m.items()}
        m.update(cvec=np.ascontiguousarray(shared["cvec"]))
        in_maps.append(m)

    from concourse.bass_utils import run_bass_kernel_spmd
    import os
    trace = os.environ.get("KERNEL_TRACE", "0") == "1"
    if trace:
        trace = _install_ntff_hook()
    try:
        res = run_bass_kernel_spmd(nc, in_maps, core_ids=list(range(NCORES)),
                                   trace=trace)
    except Exception:
        if not trace:
            raise
        import traceback
        traceback.print_exc()
        print("traced run failed; retrying without trace")
        res = run_bass_kernel_spmd(nc, in_maps, core_ids=list(range(NCORES)),
                                   trace=False)
    if trace and res.exec_time_ns is not None:
        print(f"HW exec time: {res.exec_time_ns} ns")
        kernel.last_exec_time_ns = res.exec_time_ns

    kernel.last_results = res.results
    x_full = np.zeros((N, D), np.float32)
    ea_full = np.zeros((E, D), np.float32)
    for c in range(NCORES):
        out = res.results[c]
        x_full[c * NPC:(c + 1) * NPC] = out["x_out"][:NPC]
        eid = pres[c]["orig_eid"]
        v = eid >= 0
        ea_full[eid[v]] = out["ea_out"][v]
    return x_full, ea_full
